# revision 1
# baseline (speedup 1.0000x reference)
"""GCN message-passing kernel for Trainium2, 8-core SPMD.

Model (N=8192 nodes, 64 graphs of 128 consecutive nodes):
  h   = emb[x]
  h   = relu-GCN layer 1:  D_r^-1/2 m D_c^-1/2 relu(h W1^T + b1)
  h   = relu-GCN layer 2:  D_r^-1/2 m D_c^-1/2 relu(h W2^T + b2)
  out = segment_max(h, 128-row blocks) @ Wc^T + bc

Distribution: row-shard m (1024 rows/core). Each core keeps a bf16
transposed copy of its shard resident in SBUF (m is read from HBM
exactly once), computes column-degree partials (ReduceScatter +
AllGather), the full msg1 locally, its row block of both GCN layers,
and pools/classifies its own 8 graphs. Row degrees come for free as a
ones-column appended to the layer-1 matmul rhs.
"""

import sys

for p in ("/opt/trn_rl_repo",):
    if p not in sys.path:
        sys.path.insert(0, p)

from contextlib import ExitStack

import numpy as np

import concourse.bass as bass
import concourse.mybir as mybir
import concourse.tile as tile
from concourse import bacc, bass_utils
from concourse.masks import make_identity

P = 128
N = 8192
NCORES = 8
NS = N // NCORES          # rows per core (1024)
JT = N // P               # j tiles (64)
IB = NS // P              # i blocks per core (8)
F = 128                   # hidden/emb width
C = 16                    # classes
VOCAB = 32768
G_LOCAL = IB              # graphs per core (graph == one 128-row block)

F32 = mybir.dt.float32
BF16 = mybir.dt.bfloat16
I32 = mybir.dt.int32

_CACHE = {}


def _build(reps=1):
    nc = bacc.Bacc("TRN2", target_bir_lowering=False, debug=False,
                   enable_asserts=True, num_devices=NCORES)

    m_shard = nc.dram_tensor("m_shard", [NS, N], F32, kind="ExternalInput")
    x_in = nc.dram_tensor("x_in", [N], I32, kind="ExternalInput")
    emb_in = nc.dram_tensor("emb_in", [VOCAB, F], F32, kind="ExternalInput")
    w1_in = nc.dram_tensor("w1_in", [F, F], F32, kind="ExternalInput")
    b1_in = nc.dram_tensor("b1_in", [F], F32, kind="ExternalInput")
    w2_in = nc.dram_tensor("w2_in", [F, F], F32, kind="ExternalInput")
    b2_in = nc.dram_tensor("b2_in", [F], F32, kind="ExternalInput")
    wc_in = nc.dram_tensor("wc_in", [C, F], F32, kind="ExternalInput")
    bc_in = nc.dram_tensor("bc_in", [C], F32, kind="ExternalInput")
    out_l = nc.dram_tensor("out_l", [G_LOCAL, C], F32, kind="ExternalOutput")

    with tile.TileContext(nc) as tc, ExitStack() as stack:
        consts = stack.enter_context(tc.tile_pool(name="consts", bufs=1))
        big = stack.enter_context(tc.tile_pool(name="big", bufs=1))
        dram = stack.enter_context(tc.tile_pool(name="dram", bufs=1, space="DRAM"))

        ident_bf = consts.tile([P, P], BF16)
        make_identity(nc, ident_bf)
        ident_f32 = consts.tile([P, P], F32)
        make_identity(nc, ident_f32)

        # ---- small constants -------------------------------------------
        ones_row = consts.tile([1, P], BF16)
        nc.vector.memset(ones_row[:], 1.0)
        ones_row8_f32 = consts.tile([1, G_LOCAL], F32)
        nc.vector.memset(ones_row8_f32[:], 1.0)
        b1_row = consts.tile([1, F], BF16)
        nc.gpsimd.dma_start(b1_row[:], b1_in.ap()[None, :])
        b2_row = consts.tile([1, F], BF16)
        nc.gpsimd.dma_start(b2_row[:], b2_in.ap()[None, :])
        bc_row = consts.tile([1, C], F32)
        nc.sync.dma_start(bc_row[:], bc_in.ap()[None, :])
        x_sb = consts.tile([P, JT], I32)
        nc.sync.dma_start(x_sb[:], x_in.ap().rearrange("(t p) -> p t", p=P))

        # w1T/w2T (transposed weights, bf16), wcT (f32)
        w1T = consts.tile([P, F], BF16)
        w2T = consts.tile([P, F], BF16)
        wcT = consts.tile([P, C], F32)
        with tc.tile_pool(name="wtmp", bufs=2) as wtmp, \
             tc.tile_pool(name="wpsum", bufs=2, space="PSUM") as wpsum:
            for w_in, wT in ((w1_in, w1T), (w2_in, w2T)):
                wf = wtmp.tile([F, F], F32, tag="wf")
                nc.sync.dma_start(wf[:], w_in.ap())
                wb = wtmp.tile([F, F], BF16, tag="wb")
                nc.vector.tensor_copy(wb[:], wf[:])
                ps = wpsum.tile([P, F], BF16, tag="wps")
                nc.tensor.transpose(ps[:], wb[:], ident_bf[:])
                nc.any.tensor_copy(wT[:], ps[:])
            wcf = wtmp.tile([C, F], F32, tag="wcf")
            nc.sync.dma_start(wcf[:], wc_in.ap())
            pc = wpsum.tile([P, C], F32, tag="wcps")
            nc.tensor.transpose(pc[:], wcf[:], ident_f32[:C, :C])
            nc.any.tensor_copy(wcT[:], pc[:])

        for _rep in range(reps):
            _emit_pipeline(
                nc, tc, consts, big, dram,
                m_shard, emb_in, out_l,
                ident_bf, ident_f32, ones_row, ones_row8_f32,
                b1_row, b2_row, bc_row, x_sb, w1T, w2T, wcT,
            )

    nc.compile()
    return nc


def _emit_pipeline(nc, tc, consts, big, dram, m_shard, emb_in, out_l,
                   ident_bf, ident_f32, ones_row, ones_row8_f32,
                   b1_row, b2_row, bc_row, x_sb, w1T, w2T, wcT):
    # ---- resident tensors ------------------------------------------
    mT = big.tile([P, JT, NS], BF16, tag="mT", name="mT")       # [j_in_tile, jt, i]
    hT = big.tile([P, JT, F], BF16, tag="hT", name="hT")        # [e, jt, j_in_tile]
    msg_sb = big.tile([P, JT, F + 1], BF16, tag="msg", name="msg")  # msg1' | ones
    nc.vector.memset(msg_sb[:, :, F], 1.0)

    # ---- phase A: embedding gather -> h^T --------------------------
    with tc.tile_pool(name="hwork", bufs=2) as hwork, \
         tc.tile_pool(name="hpsum", bufs=2, space="PSUM") as hpsum:
        for t in range(JT):
            h_f = hwork.tile([P, F], F32, tag="hf", name="hf")
            nc.gpsimd.indirect_dma_start(
                out=h_f[:],
                out_offset=None,
                in_=emb_in.ap(),
                in_offset=bass.IndirectOffsetOnAxis(ap=x_sb[:, t:t + 1], axis=0),
            )
            h_b = hwork.tile([P, F], BF16, tag="hb", name="hb")
            nc.vector.tensor_copy(h_b[:], h_f[:])
            ps = hpsum.tile([P, P], BF16, tag="hps", name="hps")
            nc.tensor.transpose(ps[:], h_b[:], ident_bf[:])
            nc.any.tensor_copy(hT[:, t, :], ps[:])

    # ---- phase A: m load + transpose into resident mT --------------
    with tc.tile_pool(name="slab", bufs=2) as slabp, \
         tc.tile_pool(name="tpsum", bufs=4, space="PSUM") as tpsum:
        for b in range(IB):
            slab = slabp.tile([P, N], BF16, tag="slab", name="slab")
            nc.gpsimd.dma_start(slab[:], m_shard.ap()[b * P:(b + 1) * P, :])
            for jt in range(JT):
                ps = tpsum.tile([P, P], BF16, tag="tps", name="tps")
                nc.tensor.transpose(ps[:], slab[:, jt * P:(jt + 1) * P], ident_bf[:])
                nc.any.tensor_copy(mT[:, jt, b * P:(b + 1) * P], ps[:])

    # ---- column degrees + collectives ------------------------------
    cd_acc = consts.tile([P, JT], F32, tag="cd_acc", name="cd_acc")
    nc.vector.reduce_sum(out=cd_acc[:], in_=mT[:], axis=mybir.AxisListType.X)

    cd_part = dram.tile([N], F32, tag="cd_part", name="cd_part")
    cd_loc = dram.tile([NS], F32, tag="cd_loc", name="cd_loc")
    cd_full = dram.tile([N], F32, tag="cd_full", name="cd_full", addr_space="Shared")
    nc.sync.dma_start(cd_part[:].rearrange("(t p) -> p t", p=P), cd_acc[:])
    nc.gpsimd.collective_compute(
        "ReduceScatter", mybir.AluOpType.add,
        replica_groups=[list(range(NCORES))],
        ins=[cd_part.opt()], outs=[cd_loc.opt()],
    )
    nc.gpsimd.collective_compute(
        "AllGather", mybir.AluOpType.bypass,
        replica_groups=[list(range(NCORES))],
        ins=[cd_loc.opt()], outs=[cd_full.opt()],
    )
    cd_full_sb = consts.tile([P, JT], F32, tag="cdf_sb", name="cdf_sb")
    nc.sync.dma_start(cd_full_sb[:], cd_full[:].rearrange("(t p) -> p t", p=P))
    cd_loc_sb = consts.tile([P, IB], F32, tag="cdl_sb", name="cdl_sb")
    nc.sync.dma_start(cd_loc_sb[:], cd_loc[:].rearrange("(b p) -> p b", p=P))

    s_c = consts.tile([P, JT], F32, tag="s_c", name="s_c")
    nc.scalar.sqrt(s_c[:], cd_full_sb[:])
    nc.vector.reciprocal(s_c[:], s_c[:])
    s_c_loc = consts.tile([P, IB], F32, tag="s_c_loc", name="s_c_loc")
    nc.scalar.sqrt(s_c_loc[:], cd_loc_sb[:])
    nc.vector.reciprocal(s_c_loc[:], s_c_loc[:])

    # ---- phase B: msg1' = relu(s_c * (h W1^T + 1 (x) b1)) ----------
    with tc.tile_pool(name="mpsum", bufs=2, space="PSUM") as mpsum:
        for t in range(JT):
            ps = mpsum.tile([P, F], F32, tag="mps", name="mps")
            nc.tensor.matmul(ps[:], hT[:, t, :], w1T[:], start=True, stop=False)
            nc.tensor.matmul(ps[:], ones_row[:], b1_row[:], start=False, stop=True)
            nc.scalar.activation(
                msg_sb[:, t, 0:F], ps[:],
                mybir.ActivationFunctionType.Relu,
                scale=s_c[:, t:t + 1],
            )

    # ---- phase C: t1 = m @ [msg1'|1]; h1 = s_r * t1 ----------------
    s_r = consts.tile([P, IB], F32, tag="s_r", name="s_r")
    h1_bf = consts.tile([P, IB, F], BF16, tag="h1_bf", name="h1_bf")
    with tc.tile_pool(name="c_psum", bufs=1, space="PSUM") as cpsum:
        pt1 = [cpsum.tile([P, F + 1], F32, tag=f"t1_{b}", name=f"t1_{b}")
               for b in range(IB)]
        for jt in range(JT):
            for b in range(IB):
                nc.tensor.matmul(
                    pt1[b][:], mT[:, jt, b * P:(b + 1) * P], msg_sb[:, jt, :],
                    start=(jt == 0), stop=(jt == JT - 1),
                )
        for b in range(IB):
            nc.scalar.sqrt(s_r[:, b:b + 1], pt1[b][:, F:F + 1])
            nc.vector.reciprocal(s_r[:, b:b + 1], s_r[:, b:b + 1])
            nc.scalar.activation(
                h1_bf[:, b, :], pt1[b][:, 0:F],
                mybir.ActivationFunctionType.Copy,
                scale=s_r[:, b:b + 1],
            )

    # ---- phase D: msg2' local + AllGather --------------------------
    msg2_sb = consts.tile([P, IB, F], BF16, tag="msg2", name="msg2")
    with tc.tile_pool(name="d_work", bufs=2) as dwork, \
         tc.tile_pool(name="d_psum", bufs=2, space="PSUM") as dpsum, \
         tc.tile_pool(name="d_tpsum", bufs=2, space="PSUM") as dtpsum:
        for b in range(IB):
            tps = dtpsum.tile([P, P], BF16, tag="dtps", name="dtps")
            nc.tensor.transpose(tps[:], h1_bf[:, b, :], ident_bf[:])
            h1T = dwork.tile([P, F], BF16, tag="h1T", name="h1T")
            nc.any.tensor_copy(h1T[:], tps[:])
            ps = dpsum.tile([P, F], F32, tag="dps", name="dps")
            nc.tensor.matmul(ps[:], h1T[:], w2T[:], start=True, stop=False)
            nc.tensor.matmul(ps[:], ones_row[:], b2_row[:], start=False, stop=True)
            nc.scalar.activation(
                msg2_sb[:, b, :], ps[:],
                mybir.ActivationFunctionType.Relu,
                scale=s_c_loc[:, b:b + 1],
            )

    msg2_loc = dram.tile([NS, F], BF16, tag="m2l", name="m2l")
    msg2_full = dram.tile([N, F], BF16, tag="m2f", name="m2f", addr_space="Shared")
    nc.sync.dma_start(
        msg2_loc[:].rearrange("(b p) g -> p b g", p=P), msg2_sb[:])
    nc.gpsimd.collective_compute(
        "AllGather", mybir.AluOpType.bypass,
        replica_groups=[list(range(NCORES))],
        ins=[msg2_loc.opt()], outs=[msg2_full.opt()],
    )

    # ---- phase E: t2 = m @ msg2'; h2 = s_r * t2 --------------------
    h2 = consts.tile([P, IB, F], F32, tag="h2", name="h2")
    CH = 4  # j-tiles per msg2 reload chunk
    with tc.tile_pool(name="e_work", bufs=3) as ework, \
         tc.tile_pool(name="e_psum", bufs=1, space="PSUM") as epsum:
        pt2 = [epsum.tile([P, F], F32, tag=f"t2_{b}", name=f"t2_{b}")
               for b in range(IB)]
        for jc in range(JT // CH):
            mf = ework.tile([P, CH, F], BF16, tag="mf", name="mf")
            nc.sync.dma_start(
                mf[:],
                msg2_full[:].rearrange("(a p) g -> p a g", p=P)[
                    :, jc * CH:(jc + 1) * CH, :],
            )
            for ci in range(CH):
                jt = jc * CH + ci
                for b in range(IB):
                    nc.tensor.matmul(
                        pt2[b][:], mT[:, jt, b * P:(b + 1) * P], mf[:, ci, :],
                        start=(jt == 0), stop=(jt == JT - 1),
                    )
        for b in range(IB):
            nc.scalar.activation(
                h2[:, b, :], pt2[b][:],
                mybir.ActivationFunctionType.Copy,
                scale=s_r[:, b:b + 1],
            )

    # ---- phase F: segment max + classifier -------------------------
    pooledT = consts.tile([P, G_LOCAL], F32, tag="pooledT", name="pooledT")
    out_sb = consts.tile([G_LOCAL, C], F32, tag="out_sb", name="out_sb")
    with tc.tile_pool(name="f_psum", bufs=2, space="PSUM") as fpsum, \
         tc.tile_pool(name="cls_psum", bufs=1, space="PSUM") as clspsum:
        for b in range(IB):
            tps = fpsum.tile([P, P], F32, tag="ftps", name="ftps")
            nc.tensor.transpose(tps[:], h2[:, b, :], ident_f32[:])
            nc.vector.reduce_max(
                out=pooledT[:, b:b + 1], in_=tps[:], axis=mybir.AxisListType.X)
        cps = clspsum.tile([G_LOCAL, C], F32, tag="cls", name="cls")
        nc.tensor.matmul(cps[:], pooledT[:], wcT[:], start=True, stop=False)
        nc.tensor.matmul(cps[:], ones_row8_f32[:], bc_row[:],
                         start=False, stop=True)
        nc.vector.tensor_copy(out_sb[:], cps[:])
    nc.sync.dma_start(out_l.ap(), out_sb[:])


def _get_nc():
    if "nc" not in _CACHE:
        _CACHE["nc"] = _build()
    return _CACHE["nc"]


def kernel(**inputs):
    m = np.ascontiguousarray(np.asarray(inputs["m"], dtype=np.float32))
    x = np.ascontiguousarray(np.asarray(inputs["x"]).astype(np.int32))
    emb = np.ascontiguousarray(np.asarray(inputs["emb"], dtype=np.float32))
    w1 = np.ascontiguousarray(np.asarray(inputs["w1"], dtype=np.float32))
    b1 = np.ascontiguousarray(np.asarray(inputs["b1"], dtype=np.float32))
    w2 = np.ascontiguousarray(np.asarray(inputs["w2"], dtype=np.float32))
    b2 = np.ascontiguousarray(np.asarray(inputs["b2"], dtype=np.float32))
    wc = np.ascontiguousarray(np.asarray(inputs["wc"], dtype=np.float32))
    bc = np.ascontiguousarray(np.asarray(inputs["bc"], dtype=np.float32))

    nc = _get_nc()
    in_maps = []
    for k in range(NCORES):
        in_maps.append({
            "m_shard": np.ascontiguousarray(m[k * NS:(k + 1) * NS]),
            "x_in": x, "emb_in": emb,
            "w1_in": w1, "b1_in": b1, "w2_in": w2, "b2_in": b2,
            "wc_in": wc, "bc_in": bc,
        })
    res = bass_utils.run_bass_kernel_spmd(
        nc, in_maps, core_ids=list(range(NCORES)))
    out = np.concatenate([res.results[k]["out_l"] for k in range(NCORES)], axis=0)
    return out.astype(np.float32)



# revision 21
# speedup vs baseline: 222.4398x; 222.4398x over previous
"""GCN message-passing kernel for Trainium2, 8-core SPMD.

Model (N=8192 nodes, 64 graphs of 128 consecutive nodes):
  h   = emb[x]
  h   = relu-GCN layer 1:  D_r^-1/2 m D_c^-1/2 relu(h W1^T + b1)
  h   = relu-GCN layer 2:  D_r^-1/2 m D_c^-1/2 relu(h W2^T + b2)
  out = segment_max(h, 128-row blocks) @ Wc^T + bc

Distribution: row-shard m (1024 rows/core). Each core keeps a bf16
transposed copy of its shard resident in SBUF (m is read from HBM
exactly once), computes column-degree partials incrementally during the
load (ReduceScatter + AllGather), the full msg1 locally, its row block
of both GCN layers, and pools/classifies its own 8 graphs. Row degrees
come for free as a ones-column appended to the layer-1 matmul rhs.

v1 restructure vs v0: half-slab load pipeline with per-slab column-sum
reduces (kills the monolithic 68us post-load reduce), batched embedding
gathers emitted after the slab DMA descriptor-gens (Pool engine no
longer serializes 64x1us gathers ahead of the m load), 4-wide PSUM->SBUF
transpose copies alternating DVE/Act, and phase B emitted so the PE
executes it inside the degree-collective window.
"""

import sys

for p in ("/opt/trn_rl_repo",):
    if p not in sys.path:
        sys.path.insert(0, p)

from contextlib import ExitStack

import numpy as np

import concourse.bass as bass
import concourse.mybir as mybir
import concourse.tile as tile
from concourse import bacc, bass_utils
from concourse.masks import make_identity

P = 128
N = 8192
NCORES = 8
NS = N // NCORES          # rows per core (1024)
JT = N // P               # j tiles (64)
IB = NS // P              # i blocks per core (8)
F = 128                   # hidden/emb width
C = 16                    # classes
VOCAB = 32768
G_LOCAL = IB              # graphs per core (graph == one 128-row block)
HS = 4096                 # half-slab width (f32 columns per load DMA)
NH = N // HS              # halves per slab (2)

F32 = mybir.dt.float32
BF16 = mybir.dt.bfloat16
F8 = mybir.dt.float8e4
I32 = mybir.dt.int32
I16 = mybir.dt.int16

_CACHE = {}


def _build(reps=1):
    nc = bacc.Bacc("TRN2", target_bir_lowering=False, debug=False,
                   enable_asserts=True, num_devices=NCORES,
                   dynamic_dma_scratch_size=65536)

    m_shard = nc.dram_tensor("m_shard", [NS, N], F32, kind="ExternalInput")
    x_in = nc.dram_tensor("x_in", [N], I32, kind="ExternalInput")
    emb_in = nc.dram_tensor("emb_in", [VOCAB, F], F32, kind="ExternalInput")
    w1_in = nc.dram_tensor("w1_in", [F, F], F32, kind="ExternalInput")
    b1_in = nc.dram_tensor("b1_in", [F], F32, kind="ExternalInput")
    w2_in = nc.dram_tensor("w2_in", [F, F], F32, kind="ExternalInput")
    b2_in = nc.dram_tensor("b2_in", [F], F32, kind="ExternalInput")
    wc_in = nc.dram_tensor("wc_in", [C, F], F32, kind="ExternalInput")
    bc_in = nc.dram_tensor("bc_in", [C], F32, kind="ExternalInput")
    out_l = nc.dram_tensor("out_l", [G_LOCAL, C], F32, kind="ExternalOutput")

    with tile.TileContext(nc) as tc, ExitStack() as stack:
        consts = stack.enter_context(tc.tile_pool(name="consts", bufs=1))
        big = stack.enter_context(tc.tile_pool(name="big", bufs=1))
        dram = stack.enter_context(tc.tile_pool(name="dram", bufs=1, space="DRAM"))

        ident_bf = consts.tile([P, P], BF16)
        make_identity(nc, ident_bf)
        ones_col = consts.tile([P, 1], BF16)
        nc.vector.memset(ones_col[:], 1.0)
        ident_f32 = consts.tile([P, P], F32)
        make_identity(nc, ident_f32)

        # ---- small constants -------------------------------------------
        ones_row = consts.tile([1, P], BF16)
        nc.vector.memset(ones_row[:], 1.0)
        ones_row8_f32 = consts.tile([1, G_LOCAL], F32)
        nc.vector.memset(ones_row8_f32[:], 1.0)
        b1_row = consts.tile([1, F], BF16)
        nc.gpsimd.dma_start(b1_row[:], b1_in.ap()[None, :])
        b2_row = consts.tile([1, F], BF16)
        nc.gpsimd.dma_start(b2_row[:], b2_in.ap()[None, :])
        bc_row = consts.tile([1, C], F32)
        nc.sync.dma_start(bc_row[:], bc_in.ap()[None, :])
        x_sb = consts.tile([P, JT], I32)
        nc.sync.dma_start(x_sb[:], x_in.ap().rearrange("(t p) -> p t", p=P))

        # w1T/w2T (transposed weights, bf16), wcT (f32)
        w1T = consts.tile([P, F], BF16)
        w2T = consts.tile([P, F], BF16)
        wcT = consts.tile([P, C], F32)
        with tc.tile_pool(name="wtmp", bufs=2) as wtmp, \
             tc.tile_pool(name="wpsum", bufs=2, space="PSUM") as wpsum:
            for w_in, wT in ((w1_in, w1T), (w2_in, w2T)):
                wf = wtmp.tile([F, F], F32, tag="wf")
                nc.sync.dma_start(wf[:], w_in.ap())
                wb = wtmp.tile([F, F], BF16, tag="wb")
                nc.vector.tensor_copy(wb[:], wf[:])
                ps = wpsum.tile([P, F], BF16, tag="wps")
                nc.tensor.transpose(ps[:], wb[:], ident_bf[:])
                nc.any.tensor_copy(wT[:], ps[:])
            wcf = wtmp.tile([C, F], F32, tag="wcf")
            nc.sync.dma_start(wcf[:], wc_in.ap())
            pc = wpsum.tile([P, C], F32, tag="wcps")
            nc.tensor.transpose(pc[:], wcf[:], ident_f32[:C, :C])
            nc.any.tensor_copy(wcT[:], pc[:])

        for _rep in range(reps):
            _emit_pipeline(
                nc, tc, consts, big, dram,
                m_shard, emb_in, out_l,
                ident_bf, ones_col, ident_f32, ones_row, ones_row8_f32,
                b1_row, b2_row, bc_row, x_sb, w1T, w2T, wcT,
            )

    nc.compile()
    return nc


def _emit_pipeline(nc, tc, consts, big, dram, m_shard, emb_in, out_l,
                   ident_bf, ones_col, ident_f32, ones_row, ones_row8_f32,
                   b1_row, b2_row, bc_row, x_sb, w1T, w2T, wcT):
    # ---- resident tensors ------------------------------------------
    mT = big.tile([P, JT, NS], BF16, tag="mT", name="mT")       # [j_in_tile, jt, i]
    hT = big.tile([P, JT, F], BF16, tag="hT", name="hT")        # [e, jt, j_in_tile]
    msg_sb = big.tile([P, JT, F + 1], BF16, tag="msg", name="msg")  # 64*msg1' | ones
    nc.vector.memset(msg_sb[:, :, F], 1.0)

    # ---- load phase: m half-slabs -> transpose -> mT; per-tile column
    #      sums via tiny self-contained matmuls reusing the slab blocks;
    #      embedding rows arrive via batched dma_gather chunks ----------
    cd_parts = big.tile([P, JT, IB], F32, tag="cdp", name="cdp")
    with tc.tile_pool(name="slab", bufs=2) as slabp, \
         tc.tile_pool(name="tpsum", bufs=4, space="PSUM") as tpsum, \
         tc.tile_pool(name="gath", bufs=3) as gathp, \
         tc.tile_pool(name="cdps_pool", bufs=2, space="PSUM") as cdpsp, \
         tc.tile_pool(name="gpsum", bufs=2, space="PSUM") as gpsum:

        def emit_gather(t):
            h_f = gathp.tile([P, F], F32, tag="hf", name="hf")
            nc.gpsimd.indirect_dma_start(
                out=h_f[:],
                out_offset=None,
                in_=emb_in.ap(),
                in_offset=bass.IndirectOffsetOnAxis(ap=x_sb[:, t:t + 1], axis=0),
            )
            ps = gpsum.tile([P, P], F32, tag="gps", name="gps")
            nc.tensor.transpose(ps[:], h_f[:], ident_f32[:])
            if t % 2 == 0:
                nc.vector.tensor_copy(hT[:, t, :], ps[:])
            else:
                nc.scalar.copy(hT[:, t, :], ps[:])

        for b in range(IB):
            for h in range(NH):
                slab = slabp.tile([P, HS], BF16, tag="slab", name="slab")
                nc.gpsimd.dma_start(
                    slab[:],
                    m_shard.ap()[b * P:(b + 1) * P, h * HS:(h + 1) * HS])
                jt0 = h * (HS // P)          # first j tile in this half
                cdph = cdpsp.tile([P, HS // P], F32, tag="cdph", name="cdph")
                for q in range(HS // P // 4):     # 8 quads of 4 transposes
                    ps = tpsum.tile([P, 4, P], BF16, tag="tps", name="tps")
                    for ci in range(4):
                        jj = q * 4 + ci
                        blk = slab[:, jj * P:(jj + 1) * P]
                        nc.tensor.transpose(ps[:, ci, :], blk, ident_bf[:])
                        # same stationary weights, one streamed column:
                        # this block's column sums (self-contained matmul)
                        nc.tensor.matmul(
                            cdph[:, jj:jj + 1], blk, ones_col[:],
                            start=True, stop=True)
                    jt = jt0 + q * 4
                    if q % 2 == 0:
                        nc.vector.tensor_copy(
                            mT[:, jt:jt + 4, b * P:(b + 1) * P], ps[:])
                    else:
                        nc.scalar.copy(
                            mT[:, jt:jt + 4, b * P:(b + 1) * P], ps[:])
                nc.vector.tensor_copy(cd_parts[:, jt0:jt0 + HS // P, b], cdph[:])
                hs = b * NH + h
                for t in range(hs * 4, hs * 4 + 4):
                    emit_gather(t)
        cd_acc = big.tile([P, JT], F32, tag="cd_acc", name="cd_acc")
        nc.vector.reduce_sum(out=cd_acc[:], in_=cd_parts[:],
                             axis=mybir.AxisListType.X)

    # ---- phase B: msg1'' = relu(h W1^T + 1 (x) b1)  (unscaled; the
    #      s_c scale is applied in place once the degree collective
    #      completes, so the PE fills the collective window) ----------
    with tc.tile_pool(name="b_psum", bufs=4, space="PSUM") as bpsum:
        for t in range(JT):
            psb = bpsum.tile([P, F], F32, tag="bps", name="bps")
            nc.tensor.matmul(psb[:], hT[:, t, :], w1T[:], start=True, stop=False)
            nc.tensor.matmul(psb[:], ones_row[:], b1_row[:], start=False, stop=True)
            if t % 2 == 0:
                nc.scalar.activation(
                    msg_sb[:, t, 0:F], psb[:],
                    mybir.ActivationFunctionType.Relu,
                )
            else:
                nc.vector.tensor_scalar_max(
                    out=msg_sb[:, t, 0:F], in0=psb[:], scalar1=0.0)

    # ---- column degrees + collectives ------------------------------
    cd_part = dram.tile([N], F32, tag="cd_part", name="cd_part")
    cd_loc = dram.tile([NS], F32, tag="cd_loc", name="cd_loc")
    cd_full = dram.tile([N], F32, tag="cd_full", name="cd_full", addr_space="Shared")
    nc.sync.dma_start(cd_part[:].rearrange("(t p) -> p t", p=P), cd_acc[:])
    nc.gpsimd.collective_compute(
        "ReduceScatter", mybir.AluOpType.add,
        replica_groups=[list(range(NCORES))],
        ins=[cd_part.opt()], outs=[cd_loc.opt()],
    )
    nc.gpsimd.collective_compute(
        "AllGather", mybir.AluOpType.bypass,
        replica_groups=[list(range(NCORES))],
        ins=[cd_loc.opt()], outs=[cd_full.opt()],
    )
    cd_full_sb = big.tile([P, JT], F32, tag="cdf_sb", name="cdf_sb")
    nc.sync.dma_start(cd_full_sb[:], cd_full[:].rearrange("(t p) -> p t", p=P))
    cd_loc_sb = big.tile([P, IB], F32, tag="cdl_sb", name="cdl_sb")
    nc.sync.dma_start(cd_loc_sb[:], cd_loc[:].rearrange("(b p) -> p b", p=P))

    # fp8 range trick: store 64*s_c*msg so fp8 values sit in the normal
    # range; the /64 folds into s_r (sqrt scale 4096 = 64^2).
    s_c = big.tile([P, JT], F32, tag="s_c", name="s_c")
    nc.scalar.activation(s_c[:], cd_full_sb[:],
                         mybir.ActivationFunctionType.Sqrt, scale=1.0 / 4096.0)
    nc.vector.reciprocal(s_c[:], s_c[:])
    s_c_loc = big.tile([P, IB], F32, tag="s_c_loc", name="s_c_loc")
    nc.scalar.activation(s_c_loc[:], cd_loc_sb[:],
                         mybir.ActivationFunctionType.Sqrt, scale=1.0 / 4096.0)
    nc.vector.reciprocal(s_c_loc[:], s_c_loc[:])

    # ---- s_c scale pass (in place) + phase C in two half passes.
    #      Row blocks 0..3 finish first so their msg2 half and its
    #      AllGather fire while blocks 4..7 are still accumulating. ----
    s_r = big.tile([P, IB], F32, tag="s_r", name="s_r")
    h1_bf = big.tile([P, IB, F], BF16, tag="h1_bf", name="h1_bf")
    msg2_sb = big.tile([P, IB, F], BF16, tag="msg2", name="msg2")
    HB2 = IB // 2
    msg2_loc_h = [dram.tile([NS // 2, F], BF16, tag=f"m2l{hh}", name=f"m2l{hh}")
                  for hh in range(2)]
    msg2_full_h = [dram.tile([N // 2, F], BF16, tag=f"m2f{hh}", name=f"m2f{hh}",
                             addr_space="Shared") for hh in range(2)]
    SCH = 8                   # j tiles per batched s_c scale

    def emit_phase_d(hh, dwork, dpsum, dtpsum):
        for b in range(hh * HB2, (hh + 1) * HB2):
            tps = dtpsum.tile([P, P], BF16, tag="dtps", name="dtps")
            nc.tensor.transpose(tps[:], h1_bf[:, b, :], ident_bf[:])
            h1T = dwork.tile([P, F], BF16, tag="h1T", name="h1T")
            nc.any.tensor_copy(h1T[:], tps[:])
            ps = dpsum.tile([P, F], F32, tag="dps", name="dps")
            nc.tensor.matmul(ps[:], h1T[:], w2T[:], start=True, stop=False)
            nc.tensor.matmul(ps[:], ones_row[:], b2_row[:], start=False, stop=True)
            nc.scalar.activation(
                msg2_sb[:, b, :], ps[:],
                mybir.ActivationFunctionType.Relu,
                scale=s_c_loc[:, b:b + 1],
            )
        nc.sync.dma_start(
            msg2_loc_h[hh][:].rearrange("(b p) g -> p b g", p=P),
            msg2_sb[:, hh * HB2:(hh + 1) * HB2, :])
        nc.gpsimd.collective_compute(
            "AllGather", mybir.AluOpType.bypass,
            replica_groups=[list(range(NCORES))],
            ins=[msg2_loc_h[hh].opt()], outs=[msg2_full_h[hh].opt()],
        )

    def finish_block(b, pt):
        nc.scalar.activation(s_r[:, b:b + 1], pt[:, F:F + 1],
                             mybir.ActivationFunctionType.Sqrt, scale=4096.0)
        nc.vector.reciprocal(s_r[:, b:b + 1], s_r[:, b:b + 1])
        nc.scalar.activation(
            h1_bf[:, b, :], pt[:, 0:F],
            mybir.ActivationFunctionType.Copy,
            scale=s_r[:, b:b + 1],
        )

    with tc.tile_pool(name="c_psum", bufs=1, space="PSUM") as cpsum, \
         tc.tile_pool(name="d_work", bufs=2) as dwork, \
         tc.tile_pool(name="d_psum", bufs=2, space="PSUM") as dpsum, \
         tc.tile_pool(name="d_tpsum", bufs=2, space="PSUM") as dtpsum:
        pt1 = [cpsum.tile([P, F + 1], F32, tag=f"t1_{b}", name=f"t1_{b}")
               for b in range(HB2)]
        for t in range(JT):
            if t % SCH == 0:
                a_ap, s_ap = bass.broadcast_tensor_aps(
                    msg_sb[:, t:t + SCH, 0:F], s_c[:, t:t + SCH, None])
                nc.vector.tensor_mul(msg_sb[:, t:t + SCH, 0:F], a_ap, s_ap)
            for b in range(HB2):
                nc.tensor.matmul(
                    pt1[b][:], mT[:, t, b * P:(b + 1) * P], msg_sb[:, t, :],
                    start=(t == 0), stop=(t == JT - 1),
                )
        for b in range(HB2):
            finish_block(b, pt1[b])
        emit_phase_d(0, dwork, dpsum, dtpsum)

    with tc.tile_pool(name="c2_psum", bufs=1, space="PSUM") as c2psum, \
         tc.tile_pool(name="d2_work", bufs=2) as d2work, \
         tc.tile_pool(name="d2_psum", bufs=2, space="PSUM") as d2psum, \
         tc.tile_pool(name="d2_tpsum", bufs=2, space="PSUM") as d2tpsum:
        pt1b = [c2psum.tile([P, F + 1], F32, tag=f"t1b_{b}", name=f"t1b_{b}")
                for b in range(HB2)]
        for t in range(JT):
            for b in range(HB2):
                nc.tensor.matmul(
                    pt1b[b][:], mT[:, t, (HB2 + b) * P:(HB2 + b + 1) * P],
                    msg_sb[:, t, :],
                    start=(t == 0), stop=(t == JT - 1),
                )
        for b in range(HB2):
            finish_block(HB2 + b, pt1b[b])
        emit_phase_d(1, d2work, d2psum, d2tpsum)

    # ---- phase E: t2 = m @ msg2'; h2 = s_r * t2 --------------------
    h2 = big.tile([P, IB, F], F32, tag="h2", name="h2")
    CH = 4  # j-tiles per msg2 reload chunk (per half: core-contig block)
    with tc.tile_pool(name="e_work", bufs=2) as ework, \
         tc.tile_pool(name="e_psum", bufs=1, space="PSUM") as epsum:
        pt2 = [epsum.tile([P, F], F32, tag=f"t2_{b}", name=f"t2_{b}")
               for b in range(IB)]
        n_mm = 0
        for hh in range(2):
            # half hh holds rows [c*512 .. c*512+512) of every core c, i.e.
            # global j tiles jt = c*8 + hh*4 + bb
            for c in range(NCORES):
                mf = ework.tile([P, CH, F], BF16, tag="mf", name="mf")
                nc.sync.dma_start(
                    mf[:],
                    msg2_full_h[hh][:].rearrange("(a p) g -> p a g", p=P)[
                        :, c * CH:(c + 1) * CH, :],
                )
                for bb in range(CH):
                    jt = c * IB + hh * CH + bb
                    for b in range(IB):
                        nc.tensor.matmul(
                            pt2[b][:], mT[:, jt, b * P:(b + 1) * P],
                            mf[:, bb, :],
                            start=(n_mm == 0), stop=(n_mm == JT - 1),
                        )
                    n_mm += 1
        for b in range(IB):
            nc.scalar.activation(
                h2[:, b, :], pt2[b][:],
                mybir.ActivationFunctionType.Copy,
                scale=s_r[:, b:b + 1],
            )

    # ---- phase F: segment max + classifier -------------------------
    pooledT = big.tile([P, G_LOCAL], F32, tag="pooledT", name="pooledT")
    out_sb = big.tile([G_LOCAL, C], F32, tag="out_sb", name="out_sb")
    with tc.tile_pool(name="f_psum", bufs=2, space="PSUM") as fpsum, \
         tc.tile_pool(name="cls_psum", bufs=1, space="PSUM") as clspsum:
        for b in range(IB):
            tps = fpsum.tile([P, P], F32, tag="ftps", name="ftps")
            nc.tensor.transpose(tps[:], h2[:, b, :], ident_f32[:])
            nc.vector.reduce_max(
                out=pooledT[:, b:b + 1], in_=tps[:], axis=mybir.AxisListType.X)
        cps = clspsum.tile([G_LOCAL, C], F32, tag="cls", name="cls")
        nc.tensor.matmul(cps[:], pooledT[:], wcT[:], start=True, stop=False)
        nc.tensor.matmul(cps[:], ones_row8_f32[:], bc_row[:],
                         start=False, stop=True)
        nc.vector.tensor_copy(out_sb[:], cps[:])
    nc.sync.dma_start(out_l.ap(), out_sb[:])


def _get_nc():
    if "nc" not in _CACHE:
        _CACHE["nc"] = _build()
    return _CACHE["nc"]


def kernel(**inputs):
    m = np.ascontiguousarray(np.asarray(inputs["m"], dtype=np.float32))
    x = np.ascontiguousarray(np.asarray(inputs["x"]).astype(np.int32))
    emb = np.ascontiguousarray(np.asarray(inputs["emb"], dtype=np.float32))
    w1 = np.ascontiguousarray(np.asarray(inputs["w1"], dtype=np.float32))
    b1 = np.ascontiguousarray(np.asarray(inputs["b1"], dtype=np.float32))
    w2 = np.ascontiguousarray(np.asarray(inputs["w2"], dtype=np.float32))
    b2 = np.ascontiguousarray(np.asarray(inputs["b2"], dtype=np.float32))
    wc = np.ascontiguousarray(np.asarray(inputs["wc"], dtype=np.float32))
    bc = np.ascontiguousarray(np.asarray(inputs["bc"], dtype=np.float32))

    nc = _get_nc()
    in_maps = []
    for k in range(NCORES):
        in_maps.append({
            "m_shard": np.ascontiguousarray(m[k * NS:(k + 1) * NS]),
            "x_in": x, "emb_in": emb,
            "w1_in": w1, "b1_in": b1, "w2_in": w2, "b2_in": b2,
            "wc_in": wc, "bc_in": bc,
        })
    res = bass_utils.run_bass_kernel_spmd(
        nc, in_maps, core_ids=list(range(NCORES)))
    out = np.concatenate([res.results[k]["out_l"] for k in range(NCORES)], axis=0)
    return out.astype(np.float32)
